# revision 6
# baseline (speedup 1.0000x reference)
"""Trainium2 Bass kernel for AdDiffSortLoss (v3 — interleaved pipeline).

Key change vs v2: the per-pass work is split into G row-groups and the
emission order interleaves group g's phase-1 (value recurrence, an
ACT-latency-bound serial chain) with group g-1's phase-2 (mixing,
DVE-throughput-bound).  Engine queues are in-order, so v2's structure
(all of phase 1, then all of phase 2) left the DVE idle during every
arctan wait: the isolated-pass time was ~251us against ~150us of DVE
work.  Interleaving keeps the DVE busy with mixing while ACT runs the
next group's arctans.

Phase-1 groups are F1 = rpp/G rows (large, to amortize the ~76ns DVE
fixed cost per instruction); phase 2 runs in F2-row chunks within each
group.

Second change: the GT one-hot pick + 3-way BCE is replaced by
  D = P - GTm,   GTm[f, s*8+r] = (rank_r != PI[s]) in {0,1}
  sum ln|D| = sum_all ln(1-P) + sum_pick [ln P - ln(1-P)]
with |D| computed as ts-abs_max(D, 0) (4x DVE mode) and a single ACT
Ln-accumulate per chunk.  At picks GTm=0 so D = P exactly (no
cancellation); at non-picks D = P-1 only loses P < bf16-ulp(1) ~ 2e-3
absolute per entry, harmless for the mean.

Loss: loss = -(sum ln|D|) / (B*64).
"""

import math
import numpy as np

import concourse.bass as bass
import concourse.bacc as bacc
import concourse.tile as tile
from concourse import mybir
from concourse.bass_utils import run_bass_kernel_spmd

import ml_dtypes

F32 = mybir.dt.float32
F16 = mybir.dt.float16
BF16 = mybir.dt.bfloat16

N = 8                  # row width
N_CORES = 8
BATCH = 262144
CHUNK_ROWS = 128       # F2: rows per phase-2 chunk
GROUPS = 2             # G: phase-1 groups per pass
ROWS_PER_CORE = BATCH // N_CORES   # 32768
P = 128                # partitions
RPP = ROWS_PER_CORE // P           # rows per partition (256)

A = mybir.AluOpType
AF = mybir.ActivationFunctionType
INV_PI = 1.0 / math.pi

PI = np.array([0, 2, 4, 6, 1, 3, 5, 7])  # column stored in PT slot s

# tight A-side write groups per mixing layer: (a_off, stride, n, w)
LGROUPS = {
    2: [(0, 8, 1, 4), (8, 10, 2, 6), (28, 8, 1, 4)],
    3: [(32, 8, 1, 6), (40, 8, 1, 8), (50, 8, 1, 6)],
    4: [(0, 8, 1, 6), (8, 8, 2, 8), (26, 8, 1, 6)],
    5: [(32, 8, 3, 8)],
    6: [(0, 8, 4, 8)],
    7: [(32, 8, 3, 8)],
}

MARGINS = {
    "a": [(2, [10, 3], [1, 2]), (14, None, [1, 2]),
          (40, [10, 3], [1, 2]), (48, None, [1, 2])],
    "b": [(4, [32, 2], [1, 4]), (24, [32, 2], [1, 4]),
          (16, None, [1, 2]), (46, None, [1, 2])],
}

_STOP = object()

# experiment toggles (read at build time; part of the _get_nc cache key)
MARGIN_GP = True    # margin memsets on gpsimd (False: DVE)
RANK_GP = False     # rank compare/accumulate ops on gpsimd (is_gt unsupported)
XUPD_GP = False     # phase-1 value updates (tv, x-writes) on gpsimd


def build_nc(rows_per_core=ROWS_PER_CORE, chunk_rows=CHUNK_ROWS, mix_bf16=True,
             repeats=1, parts="all", loop=None, groups=GROUPS):
    rpp = rows_per_core // P
    assert rpp * P == rows_per_core
    G = groups
    F1 = rpp // G          # phase-1 group rows
    F2 = chunk_rows        # phase-2 chunk rows
    C = F1 // F2           # chunks per group
    assert G * F1 == rpp and C * F2 == F1

    nc = bacc.Bacc("TRN2")

    pred_h = nc.declare_dram_parameter("pred", [rows_per_core, N], F32, isOutput=False)
    lab_h = nc.declare_dram_parameter("labels", [rows_per_core, N], F32, isOutput=False)
    iota_h = nc.declare_dram_parameter("iota_cr", [P, N * N + N], BF16, isOutput=False)
    out_h = nc.declare_dram_parameter("out", [P, 1], F32, isOutput=True)

    predv = pred_h[:].rearrange("(p f) n -> p f n", p=P)   # [128, rpp, 8]
    labv = lab_h[:].rearrange("(p f) n -> p f n", p=P)

    def tt(out, in0, in1, op, engine=None):
        (engine or nc.vector).tensor_tensor(out, in0, in1, op)

    with tile.TileContext(nc) as tc:
        with (
            tc.tile_pool(name="io", bufs=2) as io,
            tc.tile_pool(name="rk", bufs=2) as rk,
            tc.tile_pool(name="vt", bufs=2) as vt,
            tc.tile_pool(name="vt2", bufs=2) as vt2,
            tc.tile_pool(name="als", bufs=2) as als,
            tc.tile_pool(name="gt", bufs=2) as gtp,
            tc.tile_pool(name="pp", bufs=(1 if chunk_rows >= 128 else 2)) as pp,
            tc.tile_pool(name="lnp", bufs=1) as lnp,
            tc.tile_pool(name="mt", bufs=2) as mt,
            tc.tile_pool(name="singles", bufs=1) as singles,
        ):
            # constants
            iota_t = singles.tile([P, N * N + N], BF16, tag="iota")
            nc.sync.dma_start(out=iota_t, in_=iota_h[:])
            iota_r = iota_t[:, N * N:N * N + N]   # iota_r[p, r] = r
            total_t = singles.tile([P, 1], F32, tag="total")
            nc.vector.tensor_scalar(total_t, iota_t[:, 0:1], 0.0, None, A.mult)
            half_t = singles.tile([P, 1], F32, tag="half")
            nc.vector.memset(half_t, 0.5)
            one_t = singles.tile([P, 1], F32, tag="one")
            nc.vector.memset(one_t, 1.0)
            acc_slab = singles.tile([P, 2], F32, tag="accs")
            # all chunks' D^2 values land here; ONE Ln per pass reads it
            # (keeps Arctan/Ln ACT table-set switches to 2 per pass)
            dsq_slab = singles.tile([P, rpp, N * N], BF16, tag="dsq")

            import contextlib
            if loop is not None:
                loop_n, stag = (loop if isinstance(loop, tuple) else (loop, False))
                outer = tc.For_i(0, loop_n, staggered_reset=stag)
            else:
                outer = contextlib.nullcontext()

            with outer:
                # flat unit pipeline: unit i = (pass r, group g); phase-2 of
                # unit i-1 interleaves into phase-1 of unit i, across pass
                # boundaries too (no per-pass prologue/epilogue).
                units = [(r, g) for r in range(repeats) for g in range(G)]
                io_tiles = {}
                state = {}   # per-unit tiles shared p1 -> p2

                def emit_dma(i):
                    g = units[i][1]
                    pt_ = io.tile([P, F1, N], F32, tag="pred")
                    lt_ = io.tile([P, F1, N], F32, tag="lab")
                    nc.sync.dma_start(out=pt_, in_=predv[:, g * F1:(g + 1) * F1, :])
                    nc.sync.dma_start(out=lt_, in_=labv[:, g * F1:(g + 1) * F1, :])
                    io_tiles[i] = (pt_, lt_)

                def p1_gen(i):
                    g = units[i][1]
                    if i + 1 < len(units):
                        emit_dma(i + 1)
                    pred_t, lab_t = io_tiles.pop(i)
                    labq = rk.tile([P, F1, N], BF16, tag="labq")
                    nc.scalar.activation(labq, lab_t, AF.Identity)
                    x_a = vt.tile([P, N, F1], F16, tag="x_a")
                    x_b = vt.tile([P, N, F1], F16, tag="x_b")
                    nc.scalar.activation(
                        x_a,
                        bass.AP(tensor=pred_t.tensor, offset=pred_t.offset,
                                ap=[pred_t.ap[0], [1, N], [N, F1]]),
                        AF.Identity, scale=-10.0,
                    )
                    rank_t = rk.tile([P, F1, N], BF16, tag="rank")
                    nc.vector.tensor_copy(
                        rank_t,
                        bass.AP(tensor=iota_r.tensor, offset=iota_r.offset,
                                ap=[iota_r.ap[0], [0, F1], [1, N]]),
                    )
                    cs = rk.tile([P, F1, 28], BF16, tag="cs")
                    cs_off = [0, 0, 7, 13, 18, 22, 25, 27]  # per-shift region
                    yield

                    # is_gt is not in the Pool ISA; only the accumulates can
                    # be offloaded to gpsimd
                    rk_eng = nc.gpsimd if RANK_GP else nc.vector

                    def rank_step(s):
                        w = N - s
                        c_s = cs[:, :, cs_off[s]:cs_off[s] + w]
                        tt(c_s, labq[:, :, s:N], labq[:, :, 0:N - s], A.is_gt)
                        tt(rank_t[:, :, 0:w], rank_t[:, :, 0:w], c_s, A.add,
                           engine=rk_eng)
                        tt(rank_t[:, :, s:N], rank_t[:, :, s:N], c_s,
                           A.subtract, engine=rk_eng)

                    alfs = []
                    x_cur, x_nxt = x_a, x_b
                    for layer in range(N):
                        st = layer % 2
                        npair = (N - st) // 2

                        def slot_ap(x, base):
                            return bass.AP(
                                tensor=x.tensor, offset=x.offset + base * F1,
                                ap=[x.ap[0], [2 * F1, npair], [1, F1]],
                            )
                        a_ap = slot_ap(x_cur, st)
                        b_ap = slot_ap(x_cur, st + 1)

                        delta = vt2.tile([P, npair, F1], F16, tag="delta")
                        tt(delta, b_ap, a_ap, A.subtract)
                        at16 = vt2.tile([P, npair, F1], F16, tag="at16")
                        nc.scalar.activation(at16, delta, AF.Arctan)
                        alf = vt.tile([P, npair, F1], F16, tag=f"alf_{layer}")
                        nc.scalar.activation(alf, at16, AF.Identity,
                                             scale=INV_PI, bias=half_t)
                        alfs.append(alf)
                        yield   # DVE would wait on alf here: p2 slot

                        if layer < N - 1:
                            rank_step(layer + 1)
                            xu_eng = nc.gpsimd if XUPD_GP else nc.vector
                            tv = vt2.tile([P, npair, F1], F16, tag="tv")
                            tt(tv, alf, delta, A.mult, engine=xu_eng)
                            tt(slot_ap(x_nxt, st), b_ap, tv, A.subtract,
                               engine=xu_eng)
                            tt(slot_ap(x_nxt, st + 1), a_ap, tv, A.add,
                               engine=xu_eng)
                            if st == 1:
                                (xu_eng if XUPD_GP else nc.vector).tensor_copy(
                                    bass.AP(tensor=x_nxt.tensor, offset=x_nxt.offset,
                                            ap=[x_nxt.ap[0], [(N - 1) * F1, 2], [1, F1]]),
                                    bass.AP(tensor=x_cur.tensor, offset=x_cur.offset,
                                            ap=[x_cur.ap[0], [(N - 1) * F1, 2], [1, F1]]),
                                )
                            x_cur, x_nxt = x_nxt, x_cur
                        yield

                    # ---- deferred ACT builds (off the recurrence path) ----
                    alms = []
                    for l01 in (0, 1):
                        npair_l = (N - l01 % 2) // 2
                        alm = vt.tile([P, npair_l, F1], F16, tag=f"alm_{l01}")
                        nc.scalar.activation(alm, alfs[l01], AF.Identity,
                                             scale=-1.0, bias=one_t)
                        alms.append(alm)
                    yield
                    al2s = {}
                    for layer in range(2, N):
                        npair_l = (N - layer % 2) // 2
                        alf = alfs[layer]
                        al2 = als.tile([P, F1, npair_l, 2], BF16, tag=f"al2_{layer}")
                        nc.scalar.activation(
                            al2,
                            bass.AP(tensor=alf.tensor, offset=alf.offset,
                                    ap=[alf.ap[0], [1, F1], [F1, npair_l], [0, 2]]),
                            AF.Identity,
                        )
                        al2s[layer] = al2
                        if layer == 4:
                            yield
                    yield
                    (alf0, alm0), (alf1, alm1) = (alfs[0], alms[0]), (alfs[1], alms[1])
                    A3 = als.tile([P, F1, 3, 4], BF16, tag="A3")
                    for j, (src, p0) in enumerate([(alm0, 0), (alf0, 0),
                                                   (alf0, 1), (alm0, 1)]):
                        nc.scalar.activation(
                            bass.AP(tensor=A3.tensor, offset=A3.offset + j,
                                    ap=[A3.ap[0], [12, F1], [4, 3]]),
                            bass.AP(tensor=src.tensor, offset=src.offset + p0 * F1,
                                    ap=[src.ap[0], [1, F1], [F1, 3]]),
                            AF.Identity,
                        )
                    yield
                    B6 = als.tile([P, F1, 3, 6], BF16, tag="B6")
                    for j0, src in [(0, alf1), (2, alm1), (4, alf1)]:
                        nc.scalar.activation(
                            bass.AP(tensor=B6.tensor, offset=B6.offset + j0,
                                    ap=[B6.ap[0], [18, F1], [6, 3], [1, 2]]),
                            bass.AP(tensor=src.tensor, offset=src.offset,
                                    ap=[src.ap[0], [1, F1], [F1, 3], [0, 2]]),
                            AF.Identity,
                        )
                    yield
                    E4 = als.tile([P, F1, 4], BF16, tag="E4")
                    for j, (src, p0) in enumerate([(alf0, 0), (alm0, 0),
                                                   (alm0, 3), (alf0, 3)]):
                        nc.scalar.activation(
                            bass.AP(tensor=E4.tensor, offset=E4.offset + j,
                                    ap=[E4.ap[0], [4, F1]]),
                            bass.AP(tensor=src.tensor, offset=src.offset + p0 * F1,
                                    ap=[src.ap[0], [1, F1]]),
                            AF.Identity,
                        )
                    # GTm for the whole group (ts not_equal, 4x mode), built
                    # directly into the pass D-slab: D = pt - slab in place.
                    g0 = g * F1
                    for s in range(N):
                        nc.vector.tensor_scalar(
                            bass.AP(tensor=dsq_slab.tensor,
                                    offset=dsq_slab.offset + g0 * N * N + s * N,
                                    ap=[dsq_slab.ap[0], [N * N, F1], [1, N]]),
                            rank_t, float(PI[s]), None, A.not_equal)
                    state[i] = dict(al2s=al2s, A3=A3, B6=B6, E4=E4)
                    yield

                def p2_chunk(g, c, st_g):
                    al2s, A3, B6, E4 = (st_g["al2s"], st_g["A3"],
                                        st_g["B6"], st_g["E4"])
                    pt_a = pp.tile([P, F2, N * N], BF16, tag="pt_a")
                    pt_b = pp.tile([P, F2, N * N], BF16, tag="pt_b")
                    for pt, mkey in ((pt_a, "a"), (pt_b, "b")):
                        for off, mid, inner in MARGINS[mkey]:
                            ap = [pt.ap[0], [N * N, F2]]
                            if mid is not None:
                                ap.append(mid)
                            ap.append(inner)
                            (nc.gpsimd if MARGIN_GP else nc.vector).memset(
                                bass.AP(tensor=pt.tensor,
                                        offset=pt.offset + off, ap=ap), 0.0)
                    # analytic P2 (layers 0+1) into pt_a
                    A3k = bass.AP(tensor=A3.tensor, offset=A3.offset + c * F2 * 12,
                                  ap=[A3.ap[0], [12, F2], [4, 3], [1, 4]])
                    for j0, out_off in ((0, 32), (2, 8)):
                        B6k = bass.AP(tensor=B6.tensor,
                                      offset=B6.offset + c * F2 * 18 + j0,
                                      ap=[B6.ap[0], [18, F2], [6, 3], [1, 4]])
                        out = bass.AP(tensor=pt_a.tensor,
                                      offset=pt_a.offset + out_off,
                                      ap=[pt_a.ap[0], [N * N, F2], [10, 3],
                                          [1, 4]])
                        nc.vector.tensor_tensor(out, A3k, B6k, A.mult)
                    nc.vector.tensor_copy(
                        bass.AP(tensor=pt_a.tensor, offset=pt_a.offset,
                                ap=[pt_a.ap[0], [N * N, F2], [62, 2], [1, 2]]),
                        bass.AP(tensor=E4.tensor, offset=E4.offset + c * F2 * 4,
                                ap=[E4.ap[0], [4, F2], [2, 2], [1, 2]]),
                    )
                    yield

                    def flat_ap(pt, off, n):
                        return bass.AP(
                            tensor=pt.tensor, offset=pt.offset + off,
                            ap=[pt.ap[0], [N * N, F2], [1, n]],
                        )

                    pt_cur, pt_nxt = pt_a, pt_b
                    for layer in range(2, N):
                        st = layer % 2
                        npair = (N - st) // 2
                        wd = npair * N
                        a_off0 = 0 if st == 0 else 32
                        bdel = 32 if st == 0 else -24
                        al2k = al2s[layer]
                        A_ap = flat_ap(pt_cur, a_off0, wd)
                        B_ap = flat_ap(pt_cur, a_off0 + bdel, wd)
                        d = mt.tile([P, F2, wd], BF16, tag="d")
                        t = mt.tile([P, F2, wd], BF16, tag="t")
                        al2_v = bass.AP(
                            tensor=al2k.tensor,
                            offset=al2k.offset + c * F2 * npair * 2,
                            ap=[al2k.ap[0], [2, F2 * npair], [0, 4], [1, 2]],
                        )
                        d_v = bass.AP(
                            tensor=d.tensor, offset=d.offset,
                            ap=[d.ap[0], [N, F2 * npair], [2, 4], [1, 2]],
                        )
                        t_v2 = bass.AP(
                            tensor=t.tensor, offset=t.offset,
                            ap=[t.ap[0], [N, F2 * npair], [2, 4], [1, 2]],
                        )
                        nc.vector.tensor_tensor(d, A_ap, B_ap, A.subtract)
                        nc.vector.tensor_tensor(t_v2, al2_v, d_v, A.mult)
                        yield
                        for (aoff, gstride, ng, w) in LGROUPS[layer]:
                            toff = aoff - a_off0

                            def wap(pt, off):
                                return bass.AP(
                                    tensor=pt.tensor, offset=pt.offset + off,
                                    ap=[pt.ap[0], [N * N, F2], [gstride, ng],
                                        [1, w]])
                            tap = bass.AP(
                                tensor=t.tensor, offset=t.offset + toff,
                                ap=[t.ap[0], [wd, F2], [gstride, ng], [1, w]])
                            dap_a = bass.AP(
                                tensor=pt_cur.tensor,
                                offset=pt_cur.offset + aoff,
                                ap=[pt_cur.ap[0], [N * N, F2], [gstride, ng],
                                    [1, w]])
                            dap_b = bass.AP(
                                tensor=pt_cur.tensor,
                                offset=pt_cur.offset + aoff + bdel,
                                ap=[pt_cur.ap[0], [N * N, F2], [gstride, ng],
                                    [1, w]])
                            nc.vector.tensor_tensor(
                                wap(pt_nxt, aoff), dap_b, tap, A.add)
                            nc.vector.tensor_tensor(
                                wap(pt_nxt, aoff + bdel), dap_a, tap,
                                A.subtract)
                        if st == 1:
                            nc.vector.tensor_copy(
                                bass.AP(tensor=pt_nxt.tensor,
                                        offset=pt_nxt.offset,
                                        ap=[pt_nxt.ap[0], [N * N, F2],
                                            [56, 2], [1, N]]),
                                bass.AP(tensor=pt_cur.tensor,
                                        offset=pt_cur.offset,
                                        ap=[pt_cur.ap[0], [N * N, F2],
                                            [56, 2], [1, N]]),
                            )
                        pt_cur, pt_nxt = pt_nxt, pt_cur
                        yield

                    # ---- BCE: slab = |P - GTm| (slab pre-holds GTm) ---------
                    # (Abs is in every ACT table set: no set switch here)
                    u0 = (g * C + c) * F2
                    dslice = dsq_slab[:, u0:u0 + F2, :]
                    tt(dslice, pt_cur, dslice, A.subtract)
                    nc.scalar.activation(dslice, dslice, AF.Abs)
                    yield

                def p2_gen(i):
                    st_g = state.pop(i)
                    g = units[i][1]
                    for c in range(C):
                        yield from p2_chunk(g, c, st_g)

                def emit_ln(i):
                    # group-region Ln + accumulate into the running total.
                    # Placed right after unit i's p2 drained (during unit
                    # i+1's p1), before unit i+G overwrites this slab region.
                    g = units[i][1]
                    region = dsq_slab[:, g * F1:(g + 1) * F1, :]
                    nc.scalar.activation(
                        region.rearrange("p a b -> p (a b)"),
                        region.rearrange("p a b -> p (a b)"),
                        AF.Ln, accum_out=acc_slab[:, g:g + 1],
                    )
                    nc.vector.tensor_tensor(total_t, total_t,
                                            acc_slab[:, g:g + 1], A.add)

                # ---- interleaved driver --------------------------------
                emit_dma(0)
                prev = None
                prev_i = None
                for i in range(len(units)):
                    cur = p1_gen(i)
                    while True:
                        a_done = next(cur, _STOP) is _STOP
                        if prev is not None and parts == "all":
                            next(prev, _STOP)
                        if a_done:
                            break
                    if prev is not None and parts == "all":
                        for _ in prev:   # drain remainder of p2(i-1)
                            pass
                    if prev_i is not None and parts == "all":
                        emit_ln(prev_i)
                    if parts == "all":
                        prev, prev_i = p2_gen(i), i
                if prev is not None:
                    for _ in prev:
                        pass
                    emit_ln(prev_i)

            nc.gpsimd.dma_start(out=out_h[:], in_=total_t)

    nc.compile()
    return nc


_NC_CACHE = {}


def _get_nc(rows_per_core, chunk_rows=CHUNK_ROWS, mix_bf16=True, repeats=1,
            parts="all", loop=None):
    key = (rows_per_core, chunk_rows, mix_bf16, repeats, parts, loop)
    if key not in _NC_CACHE:
        _NC_CACHE[key] = build_nc(rows_per_core, chunk_rows, mix_bf16, repeats,
                                  parts, loop)
    return _NC_CACHE[key]


def _iota_const(mix_bf16=True):
    row = np.concatenate([np.repeat(PI, N), np.arange(N)]).astype(ml_dtypes.bfloat16)
    return np.ascontiguousarray(np.broadcast_to(row, (P, N * N + N)))


def run_on_device(pred, labels, chunk_rows=CHUNK_ROWS, mix_bf16=True, trace=False):
    rows = pred.shape[0] // N_CORES
    nc = _get_nc(rows, chunk_rows, mix_bf16)
    iota = _iota_const(mix_bf16)
    in_maps = [
        {
            "pred": np.ascontiguousarray(pred[i * rows:(i + 1) * rows]),
            "labels": np.ascontiguousarray(labels[i * rows:(i + 1) * rows]),
            "iota_cr": iota,
        }
        for i in range(N_CORES)
    ]
    res = run_bass_kernel_spmd(nc, in_maps, list(range(N_CORES)), trace=trace)
    total = np.float64(0.0)
    for r in res.results:
        total += np.asarray(r["out"], dtype=np.float64).sum()
    loss = -total / (pred.shape[0] * N * N)
    return np.float32(loss), res


def kernel(pred_scores, labels, rank_ema):
    pred = np.asarray(pred_scores, dtype=np.float32)
    lab = np.asarray(labels, dtype=np.float32)
    ema = np.asarray(rank_ema, dtype=np.float32)
    if np.any(ema != 0.0):
        order = np.argsort(-lab, axis=-1, kind="stable")
        rank_true = np.argsort(order, axis=-1, kind="stable")
        pred = (pred - ema[rank_true]).astype(np.float32)
    loss, _ = run_on_device(pred, lab)
    return np.array(loss, dtype=np.float32)
